# revision 13
# baseline (speedup 1.0000x reference)
"""Trainium2 Bass kernel for nn_DynamicFiltering.

Computation (per batch b):
  xf = frames of x                     (t, c, h, w)
  y  = LeakyReLU(conv2d(xf, w1, b1), 0.2)
  ker = conv2d(y, w2, b2)              (t, 9, h, w)
  ker = ker - mean_k(ker) + 1/45       (per-pixel kernel over K = t*3*3 = 45)
  out[c,h,w] = sum_{t,k1,k2} x_edge[c,t,h+k1-1,w+k2-1] * ker[t,k1,k2][h,w]

Sharding: 8 cores = 2 batches x 4 H-slabs of 32 rows. Each core gets
pre-padded slabs (host bakes zero padding for convs, edge padding for the
filter patches) so the device program is uniform across cores.

Per-core device program:
  - conv1/conv2 as 9 shifted-offset matmuls accumulating in PSUM (fp32r)
  - LeakyReLU as y0 + (2/3)|y0| with the 0.6 scale folded into w2 host-side
  - per-frame PE transposes bring ker into pixel-partition layout kt
  - kernel normalization + W-edge folds on DVE
  - dynamic filtering with scalar_tensor_tensor (per-partition scalar =
    per-pixel kernel value); the dj column shift is handled by three
    output accumulators plus partition-shifted kt copies (made by DMA,
    which is exempt from the engine start-partition restriction)
  - outputs transposed back via PE; the dj shift collapses to free-dim
    offsets during the merge; DMA out
"""

import numpy as np

DIM = 64
T = 5
H = 128
W = 128
SLAB = 32          # output rows per core
NCORES = 8
GH = 36            # conv grid rows: slab + 2*2 halo
GW = 130           # conv grid cols: W + 2
FR = 34            # filter rows: slab + 2 halo

_PROGRAM_CACHE = {}


def _build_program():
    import concourse.bacc as bacc
    import concourse.mybir as mybir
    from concourse.tile import TileContext

    f32 = mybir.dt.float32
    f32r = mybir.dt.float32r
    Act = mybir.ActivationFunctionType
    Alu = mybir.AluOpType

    nc = bacc.Bacc("TRN2", debug=False)

    xc_d = nc.dram_tensor("xc", [DIM, T, GH, GW], f32r, kind="ExternalInput").ap()
    xt_d = nc.dram_tensor("xt", [W, T, FR, DIM], f32, kind="ExternalInput").ap()
    w1t_d = nc.dram_tensor("w1t", [DIM, 9, DIM], f32r, kind="ExternalInput").ap()
    w2t_d = nc.dram_tensor("w2t", [DIM, 9, 9], f32r, kind="ExternalInput").ap()
    b1_d = nc.dram_tensor("b1c", [DIM, 1], f32, kind="ExternalInput").ap()
    b2_d = nc.dram_tensor("b2c", [9, 1], f32, kind="ExternalInput").ap()
    ym_d = nc.dram_tensor("ymask", [DIM, 2], f32, kind="ExternalInput").ap()
    em_d = nc.dram_tensor("emask", [W, 1], f32, kind="ExternalInput").ap()
    id_d = nc.dram_tensor("ident", [128, 128], f32, kind="ExternalInput").ap()
    out_d = nc.dram_tensor("out", [DIM, SLAB, W], f32, kind="ExternalOutput").ap()

    with TileContext(nc) as tc:
        with (
            tc.tile_pool(name="consts", bufs=1) as cpool,
            tc.tile_pool(name="xcp", bufs=2) as xcp,
            tc.tile_pool(name="yp", bufs=2) as yp,
            tc.tile_pool(name="stage", bufs=2) as stp,
            tc.tile_pool(name="kerp", bufs=1) as kerp,
            tc.tile_pool(name="ktp", bufs=1) as ktp,
            tc.tile_pool(name="accp", bufs=1) as accp,
            tc.tile_pool(name="obp", bufs=3) as obp,
        ):
            w1t_sb = cpool.tile([DIM, 9, DIM], f32r)
            nc.sync.dma_start(out=w1t_sb, in_=w1t_d)
            w2t_sb = cpool.tile([DIM, 9, 9], f32r)
            nc.sync.dma_start(out=w2t_sb, in_=w2t_d)
            b1_sb = cpool.tile([DIM, 1], f32)
            nc.sync.dma_start(out=b1_sb, in_=b1_d)
            b2_sb = cpool.tile([9, 1], f32)
            nc.sync.dma_start(out=b2_sb, in_=b2_d)
            ym_sb = cpool.tile([DIM, 2], f32)
            nc.sync.dma_start(out=ym_sb, in_=ym_d)
            em_sb = cpool.tile([W, 1], f32)
            nc.sync.dma_start(out=em_sb, in_=em_d)
            id_sb = cpool.tile([128, 128], f32)
            nc.sync.dma_start(out=id_sb, in_=id_d)
            xt_sb = cpool.tile([W, T, FR, DIM], f32)
            nc.sync.dma_start(out=xt_sb, in_=xt_d)

            kt = ktp.tile([W, SLAB, 45], f32)

            with (
                tc.tile_pool(name="ps1", bufs=2, space="PSUM") as ps1p,
                tc.tile_pool(name="ps2", bufs=2, space="PSUM") as ps2p,
                tc.tile_pool(name="pst", bufs=4, space="PSUM") as pstp,
            ):
                for f in range(T):
                    xc_f = xcp.tile([DIM, GH, GW], f32r, tag="xc")
                    nc.sync.dma_start(out=xc_f, in_=xc_d[:, f])
                    y_f = yp.tile([DIM, GH, GW], f32r, tag="y")
                    # zero-pad columns read by conv2 (memset can't take f32r)
                    u32 = mybir.dt.uint32
                    nc.vector.memset(y_f[:, 1:35, 0:1].bitcast(u32), 0)
                    nc.vector.memset(y_f[:, 1:35, 129:130].bitcast(u32), 0)

                    # conv1 + leaky relu (scaled by 0.6; compensated in w2t)
                    for rc in range(9):
                        g0 = 1 + 4 * rc
                        nr = 4 if rc < 8 else 2
                        ps = ps1p.tile([DIM, 4, W], f32, tag="ps1")
                        for idx in range(9):
                            di, dj = divmod(idx, 3)
                            rhs = xc_f[:, g0 + di - 1:g0 + di - 1 + nr, dj:dj + W]
                            nc.tensor.matmul(
                                ps[:, :nr, :],
                                lhsT=w1t_sb[:, idx, :],
                                rhs=rhs,
                                start=(idx == 0),
                                stop=(idx == 8),
                            )
                        y0 = stp.tile([DIM, 4, W], f32, tag="y0")
                        a0 = stp.tile([DIM, 4, W], f32, tag="a0")
                        nc.scalar.activation(y0[:, :nr], ps[:, :nr], Act.Identity,
                                             bias=b1_sb, scale=1.0)
                        nc.scalar.activation(a0[:, :nr], ps[:, :nr], Act.Abs,
                                             bias=b1_sb, scale=1.0)
                        # y_f = y0 + (2/3)|y0|  == (0.6*y0 + 0.4*|y0|) / 0.6
                        nc.vector.scalar_tensor_tensor(
                            out=y_f[:, g0:g0 + nr, 1:129],
                            in0=a0[:, :nr], scalar=2.0 / 3.0, in1=y0[:, :nr],
                            op0=Alu.mult, op1=Alu.add)

                    # conv2 zero-pads rows outside the image: kill y halo rows
                    # that fall outside (mask is 0 there for edge slabs)
                    nc.vector.tensor_scalar(y_f[:, 1:2, 1:129], y_f[:, 1:2, 1:129],
                                            ym_sb[:, 0:1], None, Alu.mult)
                    nc.vector.tensor_scalar(y_f[:, 34:35, 1:129], y_f[:, 34:35, 1:129],
                                            ym_sb[:, 1:2], None, Alu.mult)

                    # conv2 -> ker_f (9, slab, W); grid row = 2 + r
                    ker_f = kerp.tile([9, SLAB, W], f32, tag="kerf")
                    for rc in range(8):
                        g0 = 2 + 4 * rc
                        ps2 = ps2p.tile([9, 4, W], f32, tag="ps2")
                        for idx in range(9):
                            di, dj = divmod(idx, 3)
                            rhs = y_f[:, g0 + di - 1:g0 + di + 3, dj:dj + W]
                            nc.tensor.matmul(
                                ps2,
                                lhsT=w2t_sb[:, idx, :],
                                rhs=rhs,
                                start=(idx == 0),
                                stop=(idx == 8),
                            )
                        nc.scalar.activation(ker_f[:, 4 * rc:4 * rc + 4, :],
                                             ps2, Act.Identity, bias=b2_sb, scale=1.0)

                    # transpose ker_f (9, r, pc) -> kt[pc, r, 9f..9f+9]
                    for r in range(SLAB):
                        pst = pstp.tile([W, 9], f32, tag="pst")
                        nc.tensor.transpose(pst, ker_f[:, r, :], id_sb[:9, :9])
                        nc.scalar.copy(kt[:, r, 9 * f:9 * f + 9], pst)

            # kernel normalize: kt -= (sum/45 - 1/45)
            mean = ktp.tile([W, SLAB], f32)
            nc.vector.tensor_reduce(mean, kt, axis=mybir.AxisListType.X, op=Alu.add)
            nc.vector.tensor_scalar(mean, mean, 1.0 / 45.0, -1.0 / 45.0,
                                    Alu.mult, Alu.add)
            mean_b = mean.unsqueeze(2).broadcast_to((W, SLAB, 45))
            nc.vector.tensor_tensor(kt, kt, mean_b, Alu.subtract)

            # fold W-edge replicate-pad terms into the dj=1 kernel slot:
            #   pc=0:   m[dj=1] += m[dj=0]   (x col -1 == col 0)
            #   pc=127: m[dj=1] += m[dj=2]   (x col 128 == col 127)
            ktr = kt.rearrange("p r (t di dj) -> p r t di dj", t=T, di=3, dj=3)
            nc.vector.tensor_tensor(ktr[0:1, :, :, :, 1], ktr[0:1, :, :, :, 1],
                                    ktr[0:1, :, :, :, 0], Alu.add)
            nc.vector.scalar_tensor_tensor(
                out=ktr[96:128, :, :, :, 1],
                in0=ktr[96:128, :, :, :, 2], scalar=em_sb[96:128, :],
                in1=ktr[96:128, :, :, :, 1], op0=Alu.mult, op1=Alu.add)

            # partition-shifted copies of kt (DMA is exempt from the
            # engine start-partition restriction):
            #   kt_p1[q] = kt[q+1] (for dj=0), kt_m1[q] = kt[q-1] (for dj=2)
            kt_p1 = ktp.tile([W, SLAB, 45], f32)
            kt_m1 = ktp.tile([W, SLAB, 45], f32)
            nc.vector.memset(kt_p1[96:128], 0.0)
            nc.vector.memset(kt_m1[0:32], 0.0)
            nc.sync.dma_start(out=kt_p1[0:127], in_=kt[1:128])
            nc.sync.dma_start(out=kt_m1[1:128], in_=kt[0:127])

            # dynamic filtering into three dj-separated accumulators:
            #   acc_dj[q, r, c] += xt[q, t, r+di, c] * m_(t,di,dj)[q - dj + 1, r]
            accs = []
            for dj in range(3):
                a = accp.tile([W, SLAB, DIM], f32, name=f"acc{dj}")
                nc.vector.memset(a, 0.0)
                accs.append(a)
            ksrc = [kt_p1, kt, kt_m1]
            for t in range(T):
                for di in range(3):
                    for dj in range(3):
                        td = 9 * t + 3 * di + dj
                        for r in range(SLAB):
                            nc.vector.scalar_tensor_tensor(
                                out=accs[dj][:, r, :],
                                in0=xt_sb[:, t, r + di, :],
                                scalar=ksrc[dj][:, r, td:td + 1],
                                in1=accs[dj][:, r, :],
                                op0=Alu.mult, op1=Alu.add)

            # transpose accs (q, r, c) -> (r c, q) chunks; the dj shift is a
            # free-dim offset after transposition:
            #   out[m, pc] = T(acc1)[m, pc] + T(acc0)[m, pc-1] + T(acc2)[m, pc+1]
            a0f = accs[0].rearrange("p r c -> p (r c)")
            a1f = accs[1].rearrange("p r c -> p (r c)")
            a2f = accs[2].rearrange("p r c -> p (r c)")
            out_rcw = out_d.rearrange("c r w -> r c w")
            with tc.tile_pool(name="pso", bufs=2, space="PSUM") as psop:
                for oc in range(16):
                    sl = slice(128 * oc, 128 * (oc + 1))
                    p0 = psop.tile([128, 128], f32, tag="pso0")
                    p1 = psop.tile([128, 128], f32, tag="pso1")
                    p2 = psop.tile([128, 128], f32, tag="pso2")
                    nc.tensor.transpose(p0, a0f[:, sl], id_sb)
                    nc.tensor.transpose(p1, a1f[:, sl], id_sb)
                    nc.tensor.transpose(p2, a2f[:, sl], id_sb)
                    ob = obp.tile([128, 128], f32, tag="ob")
                    nc.vector.tensor_copy(ob, p1)
                    nc.vector.tensor_tensor(ob[:, 1:128], ob[:, 1:128],
                                            p0[:, 0:127], Alu.add)
                    nc.vector.tensor_tensor(ob[:, 0:127], ob[:, 0:127],
                                            p2[:, 1:128], Alu.add)
                    nc.sync.dma_start(out=out_rcw[2 * oc:2 * oc + 2], in_=ob)

    return nc


def _get_program():
    if "nc" not in _PROGRAM_CACHE:
        nc = _build_program()
        nc.finalize()
        _PROGRAM_CACHE["nc"] = nc
    return _PROGRAM_CACHE["nc"]


def _host_prep(x, w1, b1, w2, b2):
    """Build the 8 per-core input maps from full inputs."""
    x = np.asarray(x, dtype=np.float32)
    w1 = np.asarray(w1, dtype=np.float32)
    b1 = np.asarray(b1, dtype=np.float32)
    w2 = np.asarray(w2, dtype=np.float32)
    b2 = np.asarray(b2, dtype=np.float32)

    # w1t[ci, 3*di+dj, o] = w1[o, ci, di, dj]
    w1t = np.ascontiguousarray(w1.transpose(1, 2, 3, 0).reshape(DIM, 9, DIM))
    # w2t[ci, 3*di+dj, o] = 0.6 * w2[o, ci, di, dj]   (leaky-relu scale fold)
    w2t = np.ascontiguousarray(0.6 * w2.transpose(1, 2, 3, 0).reshape(DIM, 9, 9))
    b1c = np.ascontiguousarray(b1.reshape(DIM, 1))
    b2c = np.ascontiguousarray(b2.reshape(9, 1))
    ident = np.eye(128, dtype=np.float32)
    emask = np.zeros((W, 1), dtype=np.float32)
    emask[127, 0] = 1.0

    in_maps = []
    for core in range(NCORES):
        b, s = divmod(core, 4)
        r0 = s * SLAB
        # conv input: rows r0-2 .. r0+33 zero padded, cols -1..128 zero padded
        xc = np.zeros((DIM, T, GH, GW), dtype=np.float32)
        lo = max(0, r0 - 2)
        hi = min(H, r0 + 34)
        xc[:, :, lo - (r0 - 2):hi - (r0 - 2), 1:129] = x[b, :, :, lo:hi, :]
        # filter input, pixel-partition: xt[pc, t, r, c] = x[b, c, t, clip(r0-1+r), pc]
        rows = np.clip(np.arange(r0 - 1, r0 + 33), 0, H - 1)
        # x[b][:, :, rows, :] has shape (c, t, 34, w); -> (w, t, 34, c)
        xt = np.ascontiguousarray(x[b][:, :, rows, :].transpose(3, 1, 2, 0))
        # conv2 zero-pad mask for the y halo rows (grid rows 1 and 34)
        ymask = np.ones((DIM, 2), dtype=np.float32)
        if s == 0:
            ymask[:, 0] = 0.0
        if s == 3:
            ymask[:, 1] = 0.0
        in_maps.append({
            "xc": xc, "xt": xt, "w1t": w1t, "w2t": w2t,
            "b1c": b1c, "b2c": b2c, "ymask": ymask, "emask": emask,
            "ident": ident,
        })
    return in_maps


def kernel(x, w1, b1, w2, b2):
    from concourse.bass_utils import run_bass_kernel_spmd

    nc = _get_program()
    in_maps = _host_prep(x, w1, b1, w2, b2)
    res = run_bass_kernel_spmd(nc, in_maps, list(range(NCORES)))
    out = np.zeros((2, DIM, H, W), dtype=np.float32)
    for core in range(NCORES):
        b, s = divmod(core, 4)
        out[b, :, s * SLAB:(s + 1) * SLAB, :] = res.results[core]["out"]
    return out
